# revision 15
# baseline (speedup 1.0000x reference)
"""Distributed Bass kernel for the full attention block (LN -> QKV proj ->
16-head attention -> out proj), returning (out, kh, vh), on 8 TRN2 NeuronCores.

Sharding: token-row-parallel LN + projections (each core owns 512 of the
4096 flattened tokens), AllToAll to head-parallel attention (each core owns
2 of the 16 heads), AllToAll back to token rows for the output projection.

Self-contained: hardcodes all shapes; imports only the installed concourse
runtime (/opt/trn_rl_repo).
"""
import sys

sys.path.insert(0, "/opt/trn_rl_repo")

import numpy as np
import ml_dtypes

import concourse.bass as bass
import concourse.tile as tile
from concourse import bacc, mybir
from concourse import bass_utils
from concourse.masks import make_identity

F32 = mybir.dt.float32
BF16 = mybir.dt.bfloat16
AF = mybir.ActivationFunctionType
OP = mybir.AluOpType

W = 8            # cores
B, N, D = 2, 2048, 1024
H, HD = 16, 64
T = B * N        # 4096 flattened tokens
R = T // W       # 512 token rows per core
HC = H // W      # 2 heads per core
SCALE = HD ** -0.5
EPS = 1e-5
NQB = 512        # Nq block width
NCK = N // 128   # 16 Nk chunks per batch
NJ = T // NQB    # 8 (b, nq) blocks == A2A#2 shard count


def build(n_cores=W):
    nc = bacc.Bacc("TRN2", target_bir_lowering=False, debug=False,
                   num_devices=n_cores, enable_asserts=False)

    qr = nc.dram_tensor("qr", [R, D], F32, kind="ExternalInput")
    kr = nc.dram_tensor("kr", [R, D], F32, kind="ExternalInput")
    vr = nc.dram_tensor("vr", [R, D], F32, kind="ExternalInput")
    Wq = nc.dram_tensor("Wq", [D, D], F32, kind="ExternalInput")
    Wk = nc.dram_tensor("Wk", [D, D], F32, kind="ExternalInput")
    Wv = nc.dram_tensor("Wv", [D, D], F32, kind="ExternalInput")
    Wp = nc.dram_tensor("Wp", [D, D], F32, kind="ExternalInput")
    bq = nc.dram_tensor("bq", [D], F32, kind="ExternalInput")
    bk = nc.dram_tensor("bk", [D], F32, kind="ExternalInput")
    bv = nc.dram_tensor("bv", [D], F32, kind="ExternalInput")
    bp = nc.dram_tensor("bp", [D], F32, kind="ExternalInput")
    gq = nc.dram_tensor("gq", [D], F32, kind="ExternalInput")
    gk = nc.dram_tensor("gk", [D], F32, kind="ExternalInput")
    gv = nc.dram_tensor("gv", [D], F32, kind="ExternalInput")
    bq_ln = nc.dram_tensor("bq_ln", [D], F32, kind="ExternalInput")
    bk_ln = nc.dram_tensor("bk_ln", [D], F32, kind="ExternalInput")
    bv_ln = nc.dram_tensor("bv_ln", [D], F32, kind="ExternalInput")

    # outputs in SBUF-native layout; host unshuffles (free on CPU)
    out_r = nc.dram_tensor("out_r", [128, R // 128, D], BF16, kind="ExternalOutput")
    kh_o = nc.dram_tensor("kh_o", [128, T // 128, 128], BF16, kind="ExternalOutput")
    vh_o = nc.dram_tensor("vh_o", [HC, 128, T // 128, HD], BF16, kind="ExternalOutput")

    def bcast_ap(t):
        a = t.ap()
        return bass.AP(tensor=a.tensor, offset=a.offset, ap=[[0, 128], [1, D]])

    with tile.TileContext(nc) as tc, \
         tc.tile_pool(name="dram", bufs=1, space="DRAM") as dram:
        a2a_kv_in = dram.tile([W, 2, 128, R], BF16)
        a2a_kv_out = dram.tile([W, 2, 128, R], BF16)
        a2a_q_in = dram.tile([W, 128, R], BF16)
        a2a_q_out = dram.tile([W, 128, R], BF16)
        a2a2_in = dram.tile([NJ, 128, NQB], BF16)
        a2a2_out = dram.tile([NJ, 128, NQB], BF16)

        with tc.tile_pool(name="persist", bufs=1) as persist:
            ident = persist.tile([128, 128], BF16)
            make_identity(nc, ident)
            eps_sb = persist.tile([128, 1], F32)
            nc.vector.memset(eps_sb, EPS)
            ones_col = persist.tile([128, 1], BF16)
            nc.vector.memset(ones_col, 1.0)
            # proj biases [128, 8]
            b_sb = {}
            for name, bdram in (("bq", bq), ("bk", bk), ("bv", bv), ("bp", bp)):
                bt = persist.tile([128, D // 128], F32, tag=f"b_{name}")
                b_sb[name] = bt
                nc.sync.dma_start(bt, bdram.ap().rearrange("(m p) -> p m", p=128))

            # ---- phase 1: weights + LN + transpose + projections ---------
            w_sb = {}

            def load_weight(wname, wdram, wload, engine=None):
                wt = persist.tile([128, D // 128, D], BF16, tag=f"w_{wname}")
                w_sb[wname] = wt
                eng = engine or nc.vector
                for half in range(2):
                    wf = wload.tile([128, D // 256, D], F32, tag="wf32")
                    nc.sync.dma_start(
                        wf, wdram.ap().rearrange("(c p) d -> p c d", p=128)[
                            :, half * 4:(half + 1) * 4])
                    eng.tensor_copy(
                        out=wt[:, half * 4:(half + 1) * 4], in_=wf)

            projT = {}
            with tc.tile_pool(name="ph1", bufs=3) as ph1, \
                 tc.tile_pool(name="acts", bufs=1) as acts, \
                 tc.tile_pool(name="wload", bufs=2) as wload, \
                 tc.tile_pool(name="lngb", bufs=1) as lngb, \
                 tc.tile_pool(name="ph1ps", bufs=3, space="PSUM") as ph1ps, \
                 tc.tile_pool(name="tps", bufs=4, space="PSUM") as tps:
                load_weight("Wk", Wk, wload)
                for name, xdram, gdram, bdram, wname, bname in (
                        ("k", kr, gk, bk_ln, "Wk", "bk"),
                        ("v", vr, gv, bv_ln, "Wv", "bv"),
                        ("q", qr, gq, bq_ln, "Wq", "bq")):
                    gt = lngb.tile([128, D], F32, tag=f"g_{name}")
                    bt2 = lngb.tile([128, D], F32, tag=f"bln_{name}")
                    nc.sync.dma_start(gt, bcast_ap(gdram))
                    nc.sync.dma_start(bt2, bcast_ap(bdram))
                    xT = acts.tile([128, D // 128, R], BF16, tag=f"xT_{name}")
                    for tt in range(R // 128):
                        xt = ph1.tile([128, D], F32, tag="x_in")
                        nc.sync.dma_start(xt, xdram.ap()[tt * 128:(tt + 1) * 128])
                        stats = ph1.tile([128, 2, 6], F32, tag="stats")
                        nc.vector.bn_stats(out=stats[:, 0], in_=xt[:, 0:512])
                        nc.vector.bn_stats(out=stats[:, 1], in_=xt[:, 512:1024])
                        mv = ph1.tile([128, 2], F32, tag="mv")
                        nc.vector.bn_aggr(out=mv, in_=stats)
                        std = ph1.tile([128, 1], F32, tag="std")
                        nc.scalar.activation(out=std, in_=mv[:, 1:2], func=AF.Sqrt,
                                             bias=eps_sb, scale=1.0)
                        rsig = ph1.tile([128, 1], F32, tag="rsig")
                        nc.vector.reciprocal(out=rsig, in_=std)
                        xcg = ph1.tile([128, D], F32, tag="xcg")
                        nc.vector.scalar_tensor_tensor(
                            out=xcg, in0=xt, scalar=mv[:, 0:1], in1=gt,
                            op0=OP.subtract, op1=OP.mult)
                        xn = ph1.tile([128, D], BF16, tag="xn")
                        nc.vector.scalar_tensor_tensor(
                            out=xn, in0=xcg, scalar=rsig, in1=bt2,
                            op0=OP.mult, op1=OP.add)
                        for dc in range(D // 128):
                            tp = tps.tile([128, 128], BF16, tag="tp")
                            nc.tensor.transpose(
                                tp, xn[:, dc * 128:(dc + 1) * 128], ident)
                            nc.scalar.copy(
                                out=xT[:, dc, tt * 128:(tt + 1) * 128], in_=tp)
                    # next weight load can overlap this tensor's projections
                    if name == "k":
                        load_weight("Wv", Wv, wload)
                    elif name == "v":
                        load_weight("Wq", Wq, wload, engine=nc.gpsimd)
                    pT = acts.tile([128, D // 128, R], BF16, tag=f"pT_{name}")
                    projT[name] = pT
                    for m in range(D // 128):
                        ps = ph1ps.tile([128, R], F32, tag="proj_ps")
                        for dc in range(D // 128):
                            nc.tensor.matmul(
                                ps, w_sb[wname][:, dc, m * 128:(m + 1) * 128],
                                xT[:, dc, :], start=(dc == 0), stop=(dc == 7))
                        nc.vector.tensor_scalar(
                            out=pT[:, m, :], in0=ps,
                            scalar1=b_sb[bname][:, m:m + 1], scalar2=None,
                            op0=OP.add)
                        if name == "q":
                            nc.sync.dma_start(a2a_q_in[m], pT[:, m, :])
                        else:
                            slot = {"k": 0, "v": 1}[name]
                            nc.sync.dma_start(a2a_kv_in[m, slot], pT[:, m, :])
                    if name == "v":
                        # k+v projections done: overlap their exchange with q
                        nc.gpsimd.collective_compute(
                            "AllToAll", OP.bypass,
                            replica_groups=[list(range(n_cores))],
                            ins=[a2a_kv_in[:]], outs=[a2a_kv_out[:]])
                # Wp cast on idle gpsimd
                wt = persist.tile([128, D // 128, D], BF16, tag="w_Wp")
                w_sb["Wp"] = wt
                for half in range(2):
                    wf = wload.tile([128, D // 256, D], F32, tag="wf32")
                    nc.sync.dma_start(
                        wf, Wp.ap().rearrange("(c p) d -> p c d", p=128)[
                            :, half * 4:(half + 1) * 4])
                    nc.gpsimd.tensor_copy(
                        out=wt[:, half * 4:(half + 1) * 4], in_=wf)

            # ---- A2A #1b: q exchange (kv already in flight) --------------
            nc.gpsimd.collective_compute(
                "AllToAll", OP.bypass, replica_groups=[list(range(n_cores))],
                ins=[a2a_q_in[:]], outs=[a2a_q_out[:]])

            with tc.tile_pool(name="attn", bufs=1) as attn:
                # per-head base-0 tiles: avoids implicit PE array-mode switches
                qh_h = [attn.tile([64, T], BF16, tag=f"qh_{h}", name=f"qh_{h}")
                        for h in range(2)]
                kh_h = [attn.tile([64, T], BF16, tag=f"kh_{h}", name=f"kh_{h}")
                        for h in range(2)]
                vh_h = [attn.tile([64, T], BF16, tag=f"vh_{h}", name=f"vh_{h}")
                        for h in range(2)]
                for s in range(W):
                    for h in range(2):
                        hsl = slice(h * 64, (h + 1) * 64)
                        nc.sync.dma_start(kh_h[h][:, s * R:(s + 1) * R],
                                          a2a_kv_out[s, 0][hsl])
                        nc.sync.dma_start(vh_h[h][:, s * R:(s + 1) * R],
                                          a2a_kv_out[s, 1][hsl])
                        nc.sync.dma_start(qh_h[h][:, s * R:(s + 1) * R],
                                          a2a_q_out[s][hsl])
                # v natural + appended ones column (M=65 PV fuses softmax sums)
                v_aug = [attn.tile([128, T // 128, HD + 1], BF16,
                                   tag=f"va_{h}", name=f"va_{h}")
                         for h in range(2)]
                kh_nat = attn.tile([128, T // 128, 128], BF16)
                nc.vector.memset(v_aug[0][:, :, HD:], 1.0)
                nc.vector.memset(v_aug[1][:, :, HD:], 1.0)
                with tc.tile_pool(name="tps2", bufs=4, space="PSUM") as tps2:
                    for blk in range(T // 128):
                        for h in range(2):
                            tpv = tps2.tile([128, 64], BF16, tag="tpv")
                            nc.tensor.transpose(
                                tpv, vh_h[h][:, blk * 128:(blk + 1) * 128],
                                ident[0:64, 0:64])
                            nc.scalar.copy(out=v_aug[h][:, blk, 0:HD],
                                           in_=tpv)
                            tpk = tps2.tile([128, 64], BF16, tag="tpk")
                            nc.tensor.transpose(
                                tpk, kh_h[h][:, blk * 128:(blk + 1) * 128],
                                ident[0:64, 0:64])
                            nc.scalar.copy(
                                out=kh_nat[:, blk, h * 64:(h + 1) * 64], in_=tpk)
                nc.sync.dma_start(kh_o.ap(), kh_nat)
                for h in range(2):
                    nc.sync.dma_start(vh_o.ap()[h], v_aug[h][:, :, 0:HD])

                onT = attn.tile([128, NJ, NQB], BF16)
                with tc.tile_pool(name="attn_t", bufs=4) as attn_t, \
                     tc.tile_pool(name="s_ps_pool", bufs=2, space="PSUM") as s_ps_pool, \
                     tc.tile_pool(name="o_ps_pool", bufs=2, space="PSUM") as o_ps_pool:
                    for b in range(B):
                        for nq in range(N // NQB):
                            jblk = b * (N // NQB) + nq
                            qsl = slice(b * N + nq * NQB, b * N + (nq + 1) * NQB)
                            # per head: O^T at partitions 0:64 (pos (0,0)),
                            # softmax sums at partition 64 (pos (0,64))
                            o_psh = [o_ps_pool.tile([65, NQB], F32,
                                                    tag=f"o_ps{h}",
                                                    name=f"o_ps{h}")
                                     for h in range(2)]
                            for ckp in range(NCK // 2):
                                s_ps = [s_ps_pool.tile([128, 2 * NQB], F32,
                                                       tag="s_ps", name="s_ps")
                                        for _ in range(2)]
                                e_sb = [attn_t.tile([128, 2 * NQB], BF16,
                                                    tag=f"e_sb{h}", name=f"e_sb{h}")
                                        for h in range(2)]
                                for i in range(2):
                                    ck = 2 * ckp + i
                                    ksl = slice(b * N + ck * 128,
                                                b * N + (ck + 1) * 128)
                                    esl = slice(i * NQB, (i + 1) * NQB)
                                    for h in range(2):
                                        nc.tensor.matmul(
                                            s_ps[h][:, esl], kh_h[h][:, ksl],
                                            qh_h[h][:, qsl], start=True, stop=True)
                                for h in range(2):
                                    nc.scalar.activation(out=e_sb[h], in_=s_ps[h],
                                                         func=AF.Exp, scale=SCALE)
                                for i in range(2):
                                    ck = 2 * ckp + i
                                    st, sp = (ck == 0), (ck == NCK - 1)
                                    esl = slice(i * NQB, (i + 1) * NQB)
                                    ckb = b * NCK + ck
                                    for h in range(2):
                                        nc.tensor.matmul(
                                            o_psh[h][:, :], v_aug[h][:, ckb, :],
                                            e_sb[h][:, esl], start=st, stop=sp)
                            # normalize: per-head recip of fused sums, bcast, mult
                            for h in range(2):
                                r_sb = attn_t.tile([1, NQB], F32, tag="r_sb")
                                nc.vector.reciprocal(
                                    out=r_sb, in_=o_psh[h][64:65, :])
                                r_bc = attn_t.tile([64, NQB], F32, tag="r_bc")
                                nc.gpsimd.partition_broadcast(r_bc, r_sb)
                                nc.vector.tensor_tensor(
                                    out=onT[h * 64:(h + 1) * 64, jblk, :],
                                    in0=o_psh[h][0:64, :], in1=r_bc, op=OP.mult)
                            nc.sync.dma_start(a2a2_in[jblk], onT[:, jblk, :])

            # ---- A2A #2: head-sharded -> token-sharded -------------------
            nc.gpsimd.collective_compute(
                "AllToAll", OP.bypass, replica_groups=[list(range(n_cores))],
                ins=[a2a2_in[:]], outs=[a2a2_out[:]])

            with tc.tile_pool(name="outp", bufs=1) as outp, \
                 tc.tile_pool(name="outp_t", bufs=3) as outp_t, \
                 tc.tile_pool(name="outp_ps", bufs=3, space="PSUM") as outp_ps, \
                 tc.tile_pool(name="tps3", bufs=4, space="PSUM") as tps3:
                ofull = outp.tile([128, W, R], BF16)
                for s in range(W):
                    nc.sync.dma_start(ofull[:, s, :], a2a2_out[s])
                out_nat = outp.tile([128, R // 128, D], BF16)
                for m in range(D // 128):
                    ps = outp_ps.tile([128, R], F32, tag="out_ps")
                    for s in range(W):
                        nc.tensor.matmul(
                            ps, w_sb["Wp"][:, s, m * 128:(m + 1) * 128],
                            ofull[:, s, :], start=(s == 0), stop=(s == 7))
                    oT = outp_t.tile([128, R], BF16, tag="oT")
                    nc.vector.tensor_scalar(
                        out=oT, in0=ps, scalar1=b_sb["bp"][:, m:m + 1],
                        scalar2=None, op0=OP.add)
                    for tt in range(R // 128):
                        tpo = tps3.tile([128, 128], BF16, tag="tpo")
                        nc.tensor.transpose(
                            tpo, oT[:, tt * 128:(tt + 1) * 128], ident)
                        nc.scalar.copy(
                            out=out_nat[:, tt, m * 128:(m + 1) * 128], in_=tpo)
                nc.sync.dma_start(out_r.ap(), out_nat)

    nc.compile()
    return nc


_CACHE = {}


def _get_nc():
    if "nc" not in _CACHE:
        _CACHE["nc"] = build()
    return _CACHE["nc"]


def make_in_maps(q, k, v, gq, bq_ln, gk, bk_ln, gv, bv_ln,
                 Wq, bq, Wk, bk, Wv, bv, Wp, bp):
    qf = np.ascontiguousarray(np.asarray(q, np.float32).reshape(T, D))
    kf = np.ascontiguousarray(np.asarray(k, np.float32).reshape(T, D))
    vf = np.ascontiguousarray(np.asarray(v, np.float32).reshape(T, D))
    shared = dict(
        Wq=np.asarray(Wq, np.float32), Wk=np.asarray(Wk, np.float32),
        Wv=np.asarray(Wv, np.float32), Wp=np.asarray(Wp, np.float32),
        bq=np.asarray(bq, np.float32), bk=np.asarray(bk, np.float32),
        bv=np.asarray(bv, np.float32), bp=np.asarray(bp, np.float32),
        gq=np.asarray(gq, np.float32), gk=np.asarray(gk, np.float32),
        gv=np.asarray(gv, np.float32), bq_ln=np.asarray(bq_ln, np.float32),
        bk_ln=np.asarray(bk_ln, np.float32), bv_ln=np.asarray(bv_ln, np.float32),
    )
    in_maps = []
    for c in range(W):
        sl = slice(c * R, (c + 1) * R)
        m = dict(shared)
        m["qr"] = np.ascontiguousarray(qf[sl])
        m["kr"] = np.ascontiguousarray(kf[sl])
        m["vr"] = np.ascontiguousarray(vf[sl])
        in_maps.append(m)
    return in_maps


def gather_outputs(results):
    outs = []
    kh = np.zeros((B, H, N, HD), np.float32)
    vh = np.zeros((B, H, N, HD), np.float32)
    for c in range(W):
        # out_r [128, 4, 1024]: row r = tt*128+p of this core's 512 rows
        a = results[c]["out_r"].astype(np.float32)
        outs.append(a.transpose(1, 0, 2).reshape(R, D))
        # kh_o [128(p), 32(blk), 128(hl*64+d)]: tok = blk*128+p, b = blk//16
        a = results[c]["kh_o"].astype(np.float32)
        a = a.reshape(128, B, NCK, HC, HD)           # p, b, blk16, hl, d
        a = a.transpose(1, 3, 2, 0, 4).reshape(B, HC, N, HD)
        kh[:, HC * c:HC * (c + 1)] = a
        a = results[c]["vh_o"].astype(np.float32)    # [hl, 128, 32, 64]
        a = a.reshape(HC, 128, B, NCK, HD)           # hl, p, b, blk16, d
        a = a.transpose(2, 0, 3, 1, 4).reshape(B, HC, N, HD)
        vh[:, HC * c:HC * (c + 1)] = a
    out = np.concatenate(outs, axis=0).reshape(B, N, D)
    return out, kh, vh


def run(in_maps, trace=False):
    nc = _get_nc()
    return bass_utils.run_bass_kernel_spmd(
        nc, in_maps, core_ids=list(range(W)), trace=trace)


def kernel(**inputs):
    in_maps = make_in_maps(**inputs)
    res = run(in_maps, trace=False)
    return gather_outputs(res.results)


# revision 16
# speedup vs baseline: 1.3839x; 1.3839x over previous
"""Distributed Bass kernel for the full attention block (LN -> QKV proj ->
16-head attention -> out proj), returning (out, kh, vh), on 8 TRN2 NeuronCores.

Sharding: token-row-parallel LN + projections (each core owns 512 of the
4096 flattened tokens), AllToAll to head-parallel attention (each core owns
2 of the 16 heads), AllToAll back to token rows for the output projection.

Self-contained: hardcodes all shapes; imports only the installed concourse
runtime (/opt/trn_rl_repo).
"""
import sys

sys.path.insert(0, "/opt/trn_rl_repo")

import numpy as np
import ml_dtypes

import concourse.bass as bass
import concourse.tile as tile
from concourse import bacc, mybir
from concourse import bass_utils
from concourse.masks import make_identity

F32 = mybir.dt.float32
BF16 = mybir.dt.bfloat16
AF = mybir.ActivationFunctionType
OP = mybir.AluOpType

W = 8            # cores
B, N, D = 2, 2048, 1024
H, HD = 16, 64
T = B * N        # 4096 flattened tokens
R = T // W       # 512 token rows per core
HC = H // W      # 2 heads per core
SCALE = HD ** -0.5
EPS = 1e-5
NQB = 512        # Nq block width
NCK = N // 128   # 16 Nk chunks per batch
NJ = T // NQB    # 8 (b, nq) blocks == A2A#2 shard count


def build(n_cores=W):
    nc = bacc.Bacc("TRN2", target_bir_lowering=False, debug=False,
                   num_devices=n_cores, enable_asserts=False)

    qr = nc.dram_tensor("qr", [R, D], F32, kind="ExternalInput")
    kr = nc.dram_tensor("kr", [R, D], F32, kind="ExternalInput")
    vr = nc.dram_tensor("vr", [R, D], F32, kind="ExternalInput")
    Wq = nc.dram_tensor("Wq", [D, D], F32, kind="ExternalInput")
    Wk = nc.dram_tensor("Wk", [D, D], F32, kind="ExternalInput")
    Wv = nc.dram_tensor("Wv", [D, D], F32, kind="ExternalInput")
    Wp = nc.dram_tensor("Wp", [D, D], F32, kind="ExternalInput")
    bq = nc.dram_tensor("bq", [D], F32, kind="ExternalInput")
    bk = nc.dram_tensor("bk", [D], F32, kind="ExternalInput")
    bv = nc.dram_tensor("bv", [D], F32, kind="ExternalInput")
    bp = nc.dram_tensor("bp", [D], F32, kind="ExternalInput")
    gq = nc.dram_tensor("gq", [D], F32, kind="ExternalInput")
    gk = nc.dram_tensor("gk", [D], F32, kind="ExternalInput")
    gv = nc.dram_tensor("gv", [D], F32, kind="ExternalInput")
    bq_ln = nc.dram_tensor("bq_ln", [D], F32, kind="ExternalInput")
    bk_ln = nc.dram_tensor("bk_ln", [D], F32, kind="ExternalInput")
    bv_ln = nc.dram_tensor("bv_ln", [D], F32, kind="ExternalInput")

    # outputs in SBUF-native layout; host unshuffles (free on CPU)
    out_r = nc.dram_tensor("out_r", [128, R // 128, D], BF16, kind="ExternalOutput")
    kh_o = nc.dram_tensor("kh_o", [128, T // 128, 128], BF16, kind="ExternalOutput")
    vh_o = nc.dram_tensor("vh_o", [HC, 128, T // 128, HD], BF16, kind="ExternalOutput")

    def bcast_ap(t):
        a = t.ap()
        return bass.AP(tensor=a.tensor, offset=a.offset, ap=[[0, 128], [1, D]])

    with tile.TileContext(nc) as tc, \
         tc.tile_pool(name="dram", bufs=1, space="DRAM") as dram:
        a2a_kv_in = dram.tile([W, 2, 128, R], BF16)
        a2a_kv_out = dram.tile([W, 2, 128, R], BF16)
        a2a_q_in = dram.tile([W, 128, R], BF16)
        a2a_q_out = dram.tile([W, 128, R], BF16)
        a2a2_in = dram.tile([NJ, 128, NQB], BF16)
        a2a2_out = dram.tile([NJ, 128, NQB], BF16)

        with tc.tile_pool(name="persist", bufs=1) as persist:
            ident = persist.tile([128, 128], BF16)
            make_identity(nc, ident)
            eps_sb = persist.tile([128, 1], F32)
            nc.vector.memset(eps_sb, EPS)
            ones_col = persist.tile([128, 1], BF16)
            nc.vector.memset(ones_col, 1.0)
            # proj biases [128, 8]
            b_sb = {}
            for name, bdram in (("bq", bq), ("bk", bk), ("bv", bv), ("bp", bp)):
                bt = persist.tile([128, D // 128], F32, tag=f"b_{name}")
                b_sb[name] = bt
                nc.sync.dma_start(bt, bdram.ap().rearrange("(m p) -> p m", p=128))

            # ---- phase 1: weights + LN + transpose + projections ---------
            w_sb = {}

            def load_weight(wname, wdram, wload, engine=None):
                wt = persist.tile([128, D // 128, D], BF16, tag=f"w_{wname}")
                w_sb[wname] = wt
                eng = engine or nc.vector
                for half in range(2):
                    wf = wload.tile([128, D // 256, D], F32, tag="wf32")
                    nc.sync.dma_start(
                        wf, wdram.ap().rearrange("(c p) d -> p c d", p=128)[
                            :, half * 4:(half + 1) * 4])
                    eng.tensor_copy(
                        out=wt[:, half * 4:(half + 1) * 4], in_=wf)

            projT = {}
            with tc.tile_pool(name="ph1", bufs=3) as ph1, \
                 tc.tile_pool(name="acts", bufs=1) as acts, \
                 tc.tile_pool(name="wload", bufs=2) as wload, \
                 tc.tile_pool(name="lngb", bufs=1) as lngb, \
                 tc.tile_pool(name="ph1ps", bufs=3, space="PSUM") as ph1ps, \
                 tc.tile_pool(name="tps", bufs=4, space="PSUM") as tps:
                load_weight("Wk", Wk, wload)
                for name, xdram, gdram, bdram, wname, bname in (
                        ("k", kr, gk, bk_ln, "Wk", "bk"),
                        ("v", vr, gv, bv_ln, "Wv", "bv"),
                        ("q", qr, gq, bq_ln, "Wq", "bq")):
                    gt = lngb.tile([128, D], F32, tag=f"g_{name}")
                    bt2 = lngb.tile([128, D], F32, tag=f"bln_{name}")
                    nc.sync.dma_start(gt, bcast_ap(gdram))
                    nc.sync.dma_start(bt2, bcast_ap(bdram))
                    xT = acts.tile([128, D // 128, R], BF16, tag=f"xT_{name}")
                    for tt in range(R // 128):
                        xt = ph1.tile([128, D], F32, tag="x_in")
                        nc.sync.dma_start(xt, xdram.ap()[tt * 128:(tt + 1) * 128])
                        stats = ph1.tile([128, 2, 6], F32, tag="stats")
                        nc.vector.bn_stats(out=stats[:, 0], in_=xt[:, 0:512])
                        nc.vector.bn_stats(out=stats[:, 1], in_=xt[:, 512:1024])
                        mv = ph1.tile([128, 2], F32, tag="mv")
                        nc.vector.bn_aggr(out=mv, in_=stats)
                        std = ph1.tile([128, 1], F32, tag="std")
                        nc.scalar.activation(out=std, in_=mv[:, 1:2], func=AF.Sqrt,
                                             bias=eps_sb, scale=1.0)
                        rsig = ph1.tile([128, 1], F32, tag="rsig")
                        nc.vector.reciprocal(out=rsig, in_=std)
                        xcg = ph1.tile([128, D], F32, tag="xcg")
                        nc.vector.scalar_tensor_tensor(
                            out=xcg, in0=xt, scalar=mv[:, 0:1], in1=gt,
                            op0=OP.subtract, op1=OP.mult)
                        xn = ph1.tile([128, D], BF16, tag="xn")
                        nc.vector.scalar_tensor_tensor(
                            out=xn, in0=xcg, scalar=rsig, in1=bt2,
                            op0=OP.mult, op1=OP.add)
                        for dc in range(D // 128):
                            tp = tps.tile([128, 128], BF16, tag="tp")
                            nc.tensor.transpose(
                                tp, xn[:, dc * 128:(dc + 1) * 128], ident)
                            nc.vector.tensor_copy(
                                out=xT[:, dc, tt * 128:(tt + 1) * 128], in_=tp)
                    # next weight load can overlap this tensor's projections
                    if name == "k":
                        load_weight("Wv", Wv, wload)
                    elif name == "v":
                        load_weight("Wq", Wq, wload)
                    pT = acts.tile([128, D // 128, R], BF16, tag=f"pT_{name}")
                    projT[name] = pT
                    for m in range(D // 128):
                        ps = ph1ps.tile([128, R], F32, tag="proj_ps")
                        for dc in range(D // 128):
                            nc.tensor.matmul(
                                ps, w_sb[wname][:, dc, m * 128:(m + 1) * 128],
                                xT[:, dc, :], start=(dc == 0), stop=(dc == 7))
                        nc.vector.tensor_scalar(
                            out=pT[:, m, :], in0=ps,
                            scalar1=b_sb[bname][:, m:m + 1], scalar2=None,
                            op0=OP.add)
                        if name == "q":
                            nc.sync.dma_start(a2a_q_in[m], pT[:, m, :])
                        else:
                            slot = {"k": 0, "v": 1}[name]
                            nc.sync.dma_start(a2a_kv_in[m, slot], pT[:, m, :])
                    if name == "v":
                        # k+v projections done: overlap their exchange with q
                        nc.gpsimd.collective_compute(
                            "AllToAll", OP.bypass,
                            replica_groups=[list(range(n_cores))],
                            ins=[a2a_kv_in[:]], outs=[a2a_kv_out[:]])
                # Wp cast on idle gpsimd
                wt = persist.tile([128, D // 128, D], BF16, tag="w_Wp")
                w_sb["Wp"] = wt
                for half in range(2):
                    wf = wload.tile([128, D // 256, D], F32, tag="wf32")
                    nc.sync.dma_start(
                        wf, Wp.ap().rearrange("(c p) d -> p c d", p=128)[
                            :, half * 4:(half + 1) * 4])
                    nc.gpsimd.tensor_copy(
                        out=wt[:, half * 4:(half + 1) * 4], in_=wf)

            # ---- A2A #1b: q exchange (kv already in flight) --------------
            nc.gpsimd.collective_compute(
                "AllToAll", OP.bypass, replica_groups=[list(range(n_cores))],
                ins=[a2a_q_in[:]], outs=[a2a_q_out[:]])

            with tc.tile_pool(name="attn", bufs=1) as attn:
                # per-head base-0 tiles: avoids implicit PE array-mode switches
                qh_h = [attn.tile([64, T], BF16, tag=f"qh_{h}", name=f"qh_{h}")
                        for h in range(2)]
                kh_h = [attn.tile([64, T], BF16, tag=f"kh_{h}", name=f"kh_{h}")
                        for h in range(2)]
                vh_h = [attn.tile([64, T], BF16, tag=f"vh_{h}", name=f"vh_{h}")
                        for h in range(2)]
                for s in range(W):
                    for h in range(2):
                        hsl = slice(h * 64, (h + 1) * 64)
                        nc.sync.dma_start(kh_h[h][:, s * R:(s + 1) * R],
                                          a2a_kv_out[s, 0][hsl])
                        nc.sync.dma_start(vh_h[h][:, s * R:(s + 1) * R],
                                          a2a_kv_out[s, 1][hsl])
                        nc.sync.dma_start(qh_h[h][:, s * R:(s + 1) * R],
                                          a2a_q_out[s][hsl])
                # v natural + appended ones column (M=65 PV fuses softmax sums)
                v_aug = [attn.tile([128, T // 128, HD + 1], BF16,
                                   tag=f"va_{h}", name=f"va_{h}")
                         for h in range(2)]
                kh_nat = attn.tile([128, T // 128, 128], BF16)
                nc.vector.memset(v_aug[0][:, :, HD:], 1.0)
                nc.vector.memset(v_aug[1][:, :, HD:], 1.0)
                with tc.tile_pool(name="tps2", bufs=4, space="PSUM") as tps2:
                    for blk in range(T // 128):
                        for h in range(2):
                            tpv = tps2.tile([128, 64], BF16, tag="tpv")
                            nc.tensor.transpose(
                                tpv, vh_h[h][:, blk * 128:(blk + 1) * 128],
                                ident[0:64, 0:64])
                            nc.vector.tensor_copy(out=v_aug[h][:, blk, 0:HD],
                                                  in_=tpv)
                            tpk = tps2.tile([128, 64], BF16, tag="tpk")
                            nc.tensor.transpose(
                                tpk, kh_h[h][:, blk * 128:(blk + 1) * 128],
                                ident[0:64, 0:64])
                            nc.vector.tensor_copy(
                                out=kh_nat[:, blk, h * 64:(h + 1) * 64], in_=tpk)
                nc.sync.dma_start(kh_o.ap(), kh_nat)
                for h in range(2):
                    nc.sync.dma_start(vh_o.ap()[h], v_aug[h][:, :, 0:HD])

                onT = attn.tile([128, NJ, NQB], BF16)
                with tc.tile_pool(name="attn_t", bufs=4) as attn_t, \
                     tc.tile_pool(name="s_ps_pool", bufs=2, space="PSUM") as s_ps_pool, \
                     tc.tile_pool(name="o_ps_pool", bufs=2, space="PSUM") as o_ps_pool:
                    for b in range(B):
                        for nq in range(N // NQB):
                            jblk = b * (N // NQB) + nq
                            qsl = slice(b * N + nq * NQB, b * N + (nq + 1) * NQB)
                            # per head: O^T at partitions 0:64 (pos (0,0)),
                            # softmax sums at partition 64 (pos (0,64))
                            o_psh = [o_ps_pool.tile([65, NQB], F32,
                                                    tag=f"o_ps{h}",
                                                    name=f"o_ps{h}")
                                     for h in range(2)]
                            for ckp in range(NCK // 2):
                                s_ps = [s_ps_pool.tile([128, 2 * NQB], F32,
                                                       tag="s_ps", name="s_ps")
                                        for _ in range(2)]
                                e_sb = [attn_t.tile([128, 2 * NQB], BF16,
                                                    tag=f"e_sb{h}", name=f"e_sb{h}")
                                        for h in range(2)]
                                for i in range(2):
                                    ck = 2 * ckp + i
                                    ksl = slice(b * N + ck * 128,
                                                b * N + (ck + 1) * 128)
                                    esl = slice(i * NQB, (i + 1) * NQB)
                                    for h in range(2):
                                        nc.tensor.matmul(
                                            s_ps[h][:, esl], kh_h[h][:, ksl],
                                            qh_h[h][:, qsl], start=True, stop=True)
                                for h in range(2):
                                    nc.scalar.activation(out=e_sb[h], in_=s_ps[h],
                                                         func=AF.Exp, scale=SCALE)
                                for i in range(2):
                                    ck = 2 * ckp + i
                                    st, sp = (ck == 0), (ck == NCK - 1)
                                    esl = slice(i * NQB, (i + 1) * NQB)
                                    ckb = b * NCK + ck
                                    for h in range(2):
                                        nc.tensor.matmul(
                                            o_psh[h][:, :], v_aug[h][:, ckb, :],
                                            e_sb[h][:, esl], start=st, stop=sp)
                            # normalize: per-head recip of fused sums, bcast, mult
                            for h in range(2):
                                r_sb = attn_t.tile([1, NQB], F32, tag="r_sb")
                                nc.vector.reciprocal(
                                    out=r_sb, in_=o_psh[h][64:65, :])
                                r_bc = attn_t.tile([64, NQB], F32, tag="r_bc")
                                nc.gpsimd.partition_broadcast(r_bc, r_sb)
                                nc.vector.tensor_tensor(
                                    out=onT[h * 64:(h + 1) * 64, jblk, :],
                                    in0=o_psh[h][0:64, :], in1=r_bc, op=OP.mult)
                            nc.sync.dma_start(a2a2_in[jblk], onT[:, jblk, :])

            # ---- A2A #2: head-sharded -> token-sharded -------------------
            nc.gpsimd.collective_compute(
                "AllToAll", OP.bypass, replica_groups=[list(range(n_cores))],
                ins=[a2a2_in[:]], outs=[a2a2_out[:]])

            with tc.tile_pool(name="outp", bufs=1) as outp, \
                 tc.tile_pool(name="outp_t", bufs=3) as outp_t, \
                 tc.tile_pool(name="outp_ps", bufs=3, space="PSUM") as outp_ps, \
                 tc.tile_pool(name="tps3", bufs=4, space="PSUM") as tps3:
                ofull = outp.tile([128, W, R], BF16)
                for s in range(W):
                    nc.sync.dma_start(ofull[:, s, :], a2a2_out[s])
                out_nat = outp.tile([128, R // 128, D], BF16)
                for m in range(D // 128):
                    ps = outp_ps.tile([128, R], F32, tag="out_ps")
                    for s in range(W):
                        nc.tensor.matmul(
                            ps, w_sb["Wp"][:, s, m * 128:(m + 1) * 128],
                            ofull[:, s, :], start=(s == 0), stop=(s == 7))
                    oT = outp_t.tile([128, R], BF16, tag="oT")
                    nc.vector.tensor_scalar(
                        out=oT, in0=ps, scalar1=b_sb["bp"][:, m:m + 1],
                        scalar2=None, op0=OP.add)
                    for tt in range(R // 128):
                        tpo = tps3.tile([128, 128], BF16, tag="tpo")
                        nc.tensor.transpose(
                            tpo, oT[:, tt * 128:(tt + 1) * 128], ident)
                        nc.vector.tensor_copy(
                            out=out_nat[:, tt, m * 128:(m + 1) * 128], in_=tpo)
                nc.sync.dma_start(out_r.ap(), out_nat)

    nc.compile()
    return nc


_CACHE = {}


def _get_nc():
    if "nc" not in _CACHE:
        _CACHE["nc"] = build()
    return _CACHE["nc"]


def make_in_maps(q, k, v, gq, bq_ln, gk, bk_ln, gv, bv_ln,
                 Wq, bq, Wk, bk, Wv, bv, Wp, bp):
    qf = np.ascontiguousarray(np.asarray(q, np.float32).reshape(T, D))
    kf = np.ascontiguousarray(np.asarray(k, np.float32).reshape(T, D))
    vf = np.ascontiguousarray(np.asarray(v, np.float32).reshape(T, D))
    shared = dict(
        Wq=np.asarray(Wq, np.float32), Wk=np.asarray(Wk, np.float32),
        Wv=np.asarray(Wv, np.float32), Wp=np.asarray(Wp, np.float32),
        bq=np.asarray(bq, np.float32), bk=np.asarray(bk, np.float32),
        bv=np.asarray(bv, np.float32), bp=np.asarray(bp, np.float32),
        gq=np.asarray(gq, np.float32), gk=np.asarray(gk, np.float32),
        gv=np.asarray(gv, np.float32), bq_ln=np.asarray(bq_ln, np.float32),
        bk_ln=np.asarray(bk_ln, np.float32), bv_ln=np.asarray(bv_ln, np.float32),
    )
    in_maps = []
    for c in range(W):
        sl = slice(c * R, (c + 1) * R)
        m = dict(shared)
        m["qr"] = np.ascontiguousarray(qf[sl])
        m["kr"] = np.ascontiguousarray(kf[sl])
        m["vr"] = np.ascontiguousarray(vf[sl])
        in_maps.append(m)
    return in_maps


def gather_outputs(results):
    outs = []
    kh = np.zeros((B, H, N, HD), np.float32)
    vh = np.zeros((B, H, N, HD), np.float32)
    for c in range(W):
        # out_r [128, 4, 1024]: row r = tt*128+p of this core's 512 rows
        a = results[c]["out_r"].astype(np.float32)
        outs.append(a.transpose(1, 0, 2).reshape(R, D))
        # kh_o [128(p), 32(blk), 128(hl*64+d)]: tok = blk*128+p, b = blk//16
        a = results[c]["kh_o"].astype(np.float32)
        a = a.reshape(128, B, NCK, HC, HD)           # p, b, blk16, hl, d
        a = a.transpose(1, 3, 2, 0, 4).reshape(B, HC, N, HD)
        kh[:, HC * c:HC * (c + 1)] = a
        a = results[c]["vh_o"].astype(np.float32)    # [hl, 128, 32, 64]
        a = a.reshape(HC, 128, B, NCK, HD)           # hl, p, b, blk16, d
        a = a.transpose(2, 0, 3, 1, 4).reshape(B, HC, N, HD)
        vh[:, HC * c:HC * (c + 1)] = a
    out = np.concatenate(outs, axis=0).reshape(B, N, D)
    return out, kh, vh


def run(in_maps, trace=False):
    nc = _get_nc()
    return bass_utils.run_bass_kernel_spmd(
        nc, in_maps, core_ids=list(range(W)), trace=trace)


def kernel(**inputs):
    in_maps = make_in_maps(**inputs)
    res = run(in_maps, trace=False)
    return gather_outputs(res.results)


# revision 18
# speedup vs baseline: 1.4575x; 1.0532x over previous
"""Distributed Bass kernel for the full attention block (LN -> QKV proj ->
16-head attention -> out proj), returning (out, kh, vh), on 8 TRN2 NeuronCores.

Sharding: token-row-parallel LN + projections (each core owns 512 of the
4096 flattened tokens), AllToAll to head-parallel attention (each core owns
2 of the 16 heads), AllToAll back to token rows for the output projection.

Self-contained: hardcodes all shapes; imports only the installed concourse
runtime (/opt/trn_rl_repo).
"""
import sys

sys.path.insert(0, "/opt/trn_rl_repo")

import numpy as np
import ml_dtypes

import concourse.bass as bass
import concourse.tile as tile
from concourse import bacc, mybir
from concourse import bass_utils
from concourse.masks import make_identity

F32 = mybir.dt.float32
BF16 = mybir.dt.bfloat16
AF = mybir.ActivationFunctionType
OP = mybir.AluOpType

W = 8            # cores
B, N, D = 2, 2048, 1024
H, HD = 16, 64
T = B * N        # 4096 flattened tokens
R = T // W       # 512 token rows per core
HC = H // W      # 2 heads per core
SCALE = HD ** -0.5
EPS = 1e-5
NQB = 512        # Nq block width
NCK = N // 128   # 16 Nk chunks per batch
NJ = T // NQB    # 8 (b, nq) blocks == A2A#2 shard count


def build(n_cores=W):
    nc = bacc.Bacc("TRN2", target_bir_lowering=False, debug=False,
                   num_devices=n_cores, enable_asserts=False)

    qr = nc.dram_tensor("qr", [R, D], F32, kind="ExternalInput")
    kr = nc.dram_tensor("kr", [R, D], F32, kind="ExternalInput")
    vr = nc.dram_tensor("vr", [R, D], F32, kind="ExternalInput")
    Wq = nc.dram_tensor("Wq", [D, D], F32, kind="ExternalInput")
    Wk = nc.dram_tensor("Wk", [D, D], F32, kind="ExternalInput")
    Wv = nc.dram_tensor("Wv", [D, D], F32, kind="ExternalInput")
    Wp = nc.dram_tensor("Wp", [D, D], F32, kind="ExternalInput")
    bq = nc.dram_tensor("bq", [D], F32, kind="ExternalInput")
    bk = nc.dram_tensor("bk", [D], F32, kind="ExternalInput")
    bv = nc.dram_tensor("bv", [D], F32, kind="ExternalInput")
    bp = nc.dram_tensor("bp", [D], F32, kind="ExternalInput")
    gq = nc.dram_tensor("gq", [D], F32, kind="ExternalInput")
    gk = nc.dram_tensor("gk", [D], F32, kind="ExternalInput")
    gv = nc.dram_tensor("gv", [D], F32, kind="ExternalInput")
    bq_ln = nc.dram_tensor("bq_ln", [D], F32, kind="ExternalInput")
    bk_ln = nc.dram_tensor("bk_ln", [D], F32, kind="ExternalInput")
    bv_ln = nc.dram_tensor("bv_ln", [D], F32, kind="ExternalInput")

    # outputs in SBUF-native layout; host unshuffles (free on CPU)
    out_r = nc.dram_tensor("out_r", [128, R // 128, D], BF16, kind="ExternalOutput")
    kh_o = nc.dram_tensor("kh_o", [128, T // 128, 128], BF16, kind="ExternalOutput")
    vh_o = nc.dram_tensor("vh_o", [HC, 128, T // 128, HD], BF16, kind="ExternalOutput")

    def bcast_ap(t):
        a = t.ap()
        return bass.AP(tensor=a.tensor, offset=a.offset, ap=[[0, 128], [1, D]])

    with tile.TileContext(nc) as tc, \
         tc.tile_pool(name="dram", bufs=1, space="DRAM") as dram:
        a2a_kv_in = dram.tile([W, 2, 128, R], BF16)
        a2a_kv_out = dram.tile([W, 2, 128, R], BF16)
        a2a_q_in = dram.tile([W, 128, R], BF16)
        a2a_q_out = dram.tile([W, 128, R], BF16)
        a2a2_in = dram.tile([NJ, 128, NQB], BF16)
        a2a2_out = dram.tile([NJ, 128, NQB], BF16)

        with tc.tile_pool(name="persist", bufs=1) as persist:
            ident = persist.tile([128, 128], BF16)
            make_identity(nc, ident)
            eps_sb = persist.tile([128, 1], F32)
            nc.vector.memset(eps_sb, EPS)
            ones_col = persist.tile([128, 1], BF16)
            nc.vector.memset(ones_col, 1.0)
            # proj biases [128, 8]
            b_sb = {}
            for name, bdram in (("bq", bq), ("bk", bk), ("bv", bv), ("bp", bp)):
                bt = persist.tile([128, D // 128], F32, tag=f"b_{name}")
                b_sb[name] = bt
                nc.gpsimd.dma_start(out=bt,
                                    in_=bdram.ap().rearrange("(m p) -> p m", p=128))

            # ---- phase 1: weights + LN + transpose + projections ---------
            w_sb = {}

            def load_weight(wname, wdram, wload, engine=None):
                wt = persist.tile([128, D // 128, D], BF16, tag=f"w_{wname}")
                w_sb[wname] = wt
                eng = engine or nc.vector
                for half in range(2):
                    wf = wload.tile([128, D // 256, D], F32, tag="wf32")
                    nc.scalar.dma_start(
                        out=wf, in_=wdram.ap().rearrange("(c p) d -> p c d", p=128)[
                            :, half * 4:(half + 1) * 4])
                    eng.tensor_copy(
                        out=wt[:, half * 4:(half + 1) * 4], in_=wf)

            projT = {}
            with tc.tile_pool(name="ph1", bufs=3) as ph1, \
                 tc.tile_pool(name="acts", bufs=1) as acts, \
                 tc.tile_pool(name="wload", bufs=2) as wload, \
                 tc.tile_pool(name="lngb", bufs=1) as lngb, \
                 tc.tile_pool(name="ph1ps", bufs=3, space="PSUM") as ph1ps, \
                 tc.tile_pool(name="tps", bufs=4, space="PSUM") as tps:
                load_weight("Wk", Wk, wload)

                def ln_pass(name, xdram, gdram, bdram):
                    gt = lngb.tile([128, D], F32, tag=f"g_{name}", name=f"g_{name}")
                    bt2 = lngb.tile([128, D], F32, tag=f"bln_{name}",
                                    name=f"bln_{name}")
                    nc.gpsimd.dma_start(out=gt, in_=bcast_ap(gdram))
                    nc.gpsimd.dma_start(out=bt2, in_=bcast_ap(bdram))
                    xT = acts.tile([128, D // 128, R], BF16, tag=f"xT_{name}",
                                   name=f"xT_{name}")
                    for tt in range(R // 128):
                        xt = ph1.tile([128, D], F32, tag="x_in", name="x_in")
                        nc.scalar.dma_start(
                            out=xt, in_=xdram.ap()[tt * 128:(tt + 1) * 128])
                        stats = ph1.tile([128, 2, 6], F32, tag="stats",
                                         name="stats")
                        nc.vector.bn_stats(out=stats[:, 0], in_=xt[:, 0:512])
                        nc.vector.bn_stats(out=stats[:, 1], in_=xt[:, 512:1024])
                        mv = ph1.tile([128, 2], F32, tag="mv", name="mv")
                        nc.vector.bn_aggr(out=mv, in_=stats)
                        std = ph1.tile([128, 1], F32, tag="std", name="std")
                        nc.scalar.activation(out=std, in_=mv[:, 1:2], func=AF.Sqrt,
                                             bias=eps_sb, scale=1.0)
                        rsig = ph1.tile([128, 1], F32, tag="rsig", name="rsig")
                        nc.vector.reciprocal(out=rsig, in_=std)
                        xcg = ph1.tile([128, D], F32, tag="xcg", name="xcg")
                        nc.vector.scalar_tensor_tensor(
                            out=xcg, in0=xt, scalar=mv[:, 0:1], in1=gt,
                            op0=OP.subtract, op1=OP.mult)
                        xn = ph1.tile([128, D], BF16, tag="xn", name="xn")
                        nc.vector.scalar_tensor_tensor(
                            out=xn, in0=xcg, scalar=rsig, in1=bt2,
                            op0=OP.mult, op1=OP.add)
                        for dc in range(D // 128):
                            tp = tps.tile([128, 128], BF16, tag="tp", name="tp")
                            nc.tensor.transpose(
                                tp, xn[:, dc * 128:(dc + 1) * 128], ident)
                            nc.vector.tensor_copy(
                                out=xT[:, dc, tt * 128:(tt + 1) * 128], in_=tp)
                    return xT

                def proj_pass(name, xT, wname, bname):
                    pT = acts.tile([128, D // 128, R], BF16, tag=f"pT_{name}",
                                   name=f"pT_{name}")
                    for m in range(D // 128):
                        ps = ph1ps.tile([128, R], F32, tag="proj_ps",
                                        name="proj_ps")
                        for dc in range(D // 128):
                            nc.tensor.matmul(
                                ps, w_sb[wname][:, dc, m * 128:(m + 1) * 128],
                                xT[:, dc, :], start=(dc == 0), stop=(dc == 7))
                        nc.vector.tensor_scalar(
                            out=pT[:, m, :], in0=ps,
                            scalar1=b_sb[bname][:, m:m + 1], scalar2=None,
                            op0=OP.add)
                        if name == "q":
                            nc.sync.dma_start(a2a_q_in[m], pT[:, m, :])
                        else:
                            slot = {"k": 0, "v": 1}[name]
                            nc.sync.dma_start(a2a_kv_in[m, slot], pT[:, m, :])

                xT_k = ln_pass("k", kr, gk, bk_ln)
                load_weight("Wv", Wv, wload)
                xT_v = ln_pass("v", vr, gv, bv_ln)
                proj_pass("k", xT_k, "Wk", "bk")
                proj_pass("v", xT_v, "Wv", "bv")
                # k+v projections done: overlap their exchange with q's LN+proj
                nc.gpsimd.collective_compute(
                    "AllToAll", OP.bypass,
                    replica_groups=[list(range(n_cores))],
                    ins=[a2a_kv_in[:]], outs=[a2a_kv_out[:]])
                load_weight("Wq", Wq, wload)
                xT_q = ln_pass("q", qr, gq, bq_ln)
                proj_pass("q", xT_q, "Wq", "bq")
                # Wp cast on idle gpsimd
                wt = persist.tile([128, D // 128, D], BF16, tag="w_Wp")
                w_sb["Wp"] = wt
                for half in range(2):
                    wf = wload.tile([128, D // 256, D], F32, tag="wf32")
                    nc.scalar.dma_start(
                        out=wf, in_=Wp.ap().rearrange("(c p) d -> p c d", p=128)[
                            :, half * 4:(half + 1) * 4])
                    nc.gpsimd.tensor_copy(
                        out=wt[:, half * 4:(half + 1) * 4], in_=wf)

            # ---- A2A #1b: q exchange (kv already in flight) --------------
            nc.gpsimd.collective_compute(
                "AllToAll", OP.bypass, replica_groups=[list(range(n_cores))],
                ins=[a2a_q_in[:]], outs=[a2a_q_out[:]])

            with tc.tile_pool(name="attn", bufs=1) as attn:
                # per-head base-0 tiles: avoids implicit PE array-mode switches
                qh_h = [attn.tile([64, T], BF16, tag=f"qh_{h}", name=f"qh_{h}")
                        for h in range(2)]
                kh_h = [attn.tile([64, T], BF16, tag=f"kh_{h}", name=f"kh_{h}")
                        for h in range(2)]
                vh_h = [attn.tile([64, T], BF16, tag=f"vh_{h}", name=f"vh_{h}")
                        for h in range(2)]
                for s in range(W):
                    for h in range(2):
                        hsl = slice(h * 64, (h + 1) * 64)
                        nc.sync.dma_start(kh_h[h][:, s * R:(s + 1) * R],
                                          a2a_kv_out[s, 0][hsl])
                        nc.scalar.dma_start(out=vh_h[h][:, s * R:(s + 1) * R],
                                            in_=a2a_kv_out[s, 1][hsl])
                        nc.sync.dma_start(qh_h[h][:, s * R:(s + 1) * R],
                                          a2a_q_out[s][hsl])
                # v natural + appended ones column (M=65 PV fuses softmax sums)
                v_aug = [attn.tile([128, T // 128, HD + 1], BF16,
                                   tag=f"va_{h}", name=f"va_{h}")
                         for h in range(2)]
                kh_nat = attn.tile([128, T // 128, 128], BF16)
                nc.vector.memset(v_aug[0][:, :, HD:], 1.0)
                nc.vector.memset(v_aug[1][:, :, HD:], 1.0)
                with tc.tile_pool(name="tps2", bufs=4, space="PSUM") as tps2:
                    for blk in range(T // 128):
                        for h in range(2):
                            tpv = tps2.tile([128, 64], BF16, tag="tpv")
                            nc.tensor.transpose(
                                tpv, vh_h[h][:, blk * 128:(blk + 1) * 128],
                                ident[0:64, 0:64])
                            nc.vector.tensor_copy(out=v_aug[h][:, blk, 0:HD],
                                                  in_=tpv)
                            tpk = tps2.tile([128, 64], BF16, tag="tpk")
                            nc.tensor.transpose(
                                tpk, kh_h[h][:, blk * 128:(blk + 1) * 128],
                                ident[0:64, 0:64])
                            nc.vector.tensor_copy(
                                out=kh_nat[:, blk, h * 64:(h + 1) * 64], in_=tpk)
                nc.gpsimd.dma_start(out=kh_o.ap(), in_=kh_nat)
                for h in range(2):
                    nc.gpsimd.dma_start(out=vh_o.ap()[h],
                                        in_=v_aug[h][:, :, 0:HD])

                onT = attn.tile([128, NJ, NQB], BF16)
                with tc.tile_pool(name="attn_t", bufs=4) as attn_t, \
                     tc.tile_pool(name="s_ps_pool", bufs=3, space="PSUM") as s_ps_pool, \
                     tc.tile_pool(name="o_ps_pool", bufs=1, space="PSUM") as o_ps_pool:
                    for b in range(B):
                        for nq in range(N // NQB):
                            jblk = b * (N // NQB) + nq
                            qsl = slice(b * N + nq * NQB, b * N + (nq + 1) * NQB)
                            # per head: O^T at partitions 0:64 (pos (0,0)),
                            # softmax sums at partition 64 (pos (0,64))
                            o_psh = [o_ps_pool.tile([65, NQB], F32,
                                                    tag=f"o_ps{h}",
                                                    name=f"o_ps{h}")
                                     for h in range(2)]
                            for ckp in range(NCK // 2):
                                s_ps = [s_ps_pool.tile([128, 2 * NQB], F32,
                                                       tag="s_ps", name="s_ps")
                                        for _ in range(2)]
                                e_sb = [attn_t.tile([128, 2 * NQB], BF16,
                                                    tag=f"e_sb{h}", name=f"e_sb{h}")
                                        for h in range(2)]
                                for i in range(2):
                                    ck = 2 * ckp + i
                                    ksl = slice(b * N + ck * 128,
                                                b * N + (ck + 1) * 128)
                                    esl = slice(i * NQB, (i + 1) * NQB)
                                    for h in range(2):
                                        nc.tensor.matmul(
                                            s_ps[h][:, esl], kh_h[h][:, ksl],
                                            qh_h[h][:, qsl], start=True, stop=True)
                                for h in range(2):
                                    nc.scalar.activation(out=e_sb[h], in_=s_ps[h],
                                                         func=AF.Exp, scale=SCALE)
                                for i in range(2):
                                    ck = 2 * ckp + i
                                    st, sp = (ck == 0), (ck == NCK - 1)
                                    esl = slice(i * NQB, (i + 1) * NQB)
                                    ckb = b * NCK + ck
                                    for h in range(2):
                                        nc.tensor.matmul(
                                            o_psh[h][:, :], v_aug[h][:, ckb, :],
                                            e_sb[h][:, esl], start=st, stop=sp)
                            # normalize: per-head recip of fused sums, bcast, mult
                            for h in range(2):
                                r_sb = attn_t.tile([1, NQB], F32, tag="r_sb")
                                nc.vector.reciprocal(
                                    out=r_sb, in_=o_psh[h][64:65, :])
                                r_bc = attn_t.tile([64, NQB], F32, tag="r_bc")
                                nc.gpsimd.partition_broadcast(r_bc, r_sb)
                                nc.vector.tensor_tensor(
                                    out=onT[h * 64:(h + 1) * 64, jblk, :],
                                    in0=o_psh[h][0:64, :], in1=r_bc, op=OP.mult)
                            nc.sync.dma_start(a2a2_in[jblk], onT[:, jblk, :])

            # ---- A2A #2: head-sharded -> token-sharded -------------------
            nc.gpsimd.collective_compute(
                "AllToAll", OP.bypass, replica_groups=[list(range(n_cores))],
                ins=[a2a2_in[:]], outs=[a2a2_out[:]])

            with tc.tile_pool(name="outp", bufs=1) as outp, \
                 tc.tile_pool(name="outp_t", bufs=3) as outp_t, \
                 tc.tile_pool(name="outp_ps", bufs=3, space="PSUM") as outp_ps, \
                 tc.tile_pool(name="tps3", bufs=4, space="PSUM") as tps3:
                ofull = outp.tile([128, W, R], BF16)
                for s in range(W):
                    nc.sync.dma_start(ofull[:, s, :], a2a2_out[s])
                out_nat = outp.tile([128, R // 128, D], BF16)
                for m in range(D // 128):
                    ps = outp_ps.tile([128, R], F32, tag="out_ps")
                    for s in range(W):
                        nc.tensor.matmul(
                            ps, w_sb["Wp"][:, s, m * 128:(m + 1) * 128],
                            ofull[:, s, :], start=(s == 0), stop=(s == 7))
                    oT = outp_t.tile([128, R], BF16, tag="oT")
                    nc.vector.tensor_scalar(
                        out=oT, in0=ps, scalar1=b_sb["bp"][:, m:m + 1],
                        scalar2=None, op0=OP.add)
                    for tt in range(R // 128):
                        tpo = tps3.tile([128, 128], BF16, tag="tpo")
                        nc.tensor.transpose(
                            tpo, oT[:, tt * 128:(tt + 1) * 128], ident)
                        nc.vector.tensor_copy(
                            out=out_nat[:, tt, m * 128:(m + 1) * 128], in_=tpo)
                nc.sync.dma_start(out_r.ap(), out_nat)

    nc.compile()
    return nc


_CACHE = {}


def _get_nc():
    if "nc" not in _CACHE:
        _CACHE["nc"] = build()
    return _CACHE["nc"]


def make_in_maps(q, k, v, gq, bq_ln, gk, bk_ln, gv, bv_ln,
                 Wq, bq, Wk, bk, Wv, bv, Wp, bp):
    qf = np.ascontiguousarray(np.asarray(q, np.float32).reshape(T, D))
    kf = np.ascontiguousarray(np.asarray(k, np.float32).reshape(T, D))
    vf = np.ascontiguousarray(np.asarray(v, np.float32).reshape(T, D))
    shared = dict(
        Wq=np.asarray(Wq, np.float32), Wk=np.asarray(Wk, np.float32),
        Wv=np.asarray(Wv, np.float32), Wp=np.asarray(Wp, np.float32),
        bq=np.asarray(bq, np.float32), bk=np.asarray(bk, np.float32),
        bv=np.asarray(bv, np.float32), bp=np.asarray(bp, np.float32),
        gq=np.asarray(gq, np.float32), gk=np.asarray(gk, np.float32),
        gv=np.asarray(gv, np.float32), bq_ln=np.asarray(bq_ln, np.float32),
        bk_ln=np.asarray(bk_ln, np.float32), bv_ln=np.asarray(bv_ln, np.float32),
    )
    in_maps = []
    for c in range(W):
        sl = slice(c * R, (c + 1) * R)
        m = dict(shared)
        m["qr"] = np.ascontiguousarray(qf[sl])
        m["kr"] = np.ascontiguousarray(kf[sl])
        m["vr"] = np.ascontiguousarray(vf[sl])
        in_maps.append(m)
    return in_maps


def gather_outputs(results):
    outs = []
    kh = np.zeros((B, H, N, HD), np.float32)
    vh = np.zeros((B, H, N, HD), np.float32)
    for c in range(W):
        # out_r [128, 4, 1024]: row r = tt*128+p of this core's 512 rows
        a = results[c]["out_r"].astype(np.float32)
        outs.append(a.transpose(1, 0, 2).reshape(R, D))
        # kh_o [128(p), 32(blk), 128(hl*64+d)]: tok = blk*128+p, b = blk//16
        a = results[c]["kh_o"].astype(np.float32)
        a = a.reshape(128, B, NCK, HC, HD)           # p, b, blk16, hl, d
        a = a.transpose(1, 3, 2, 0, 4).reshape(B, HC, N, HD)
        kh[:, HC * c:HC * (c + 1)] = a
        a = results[c]["vh_o"].astype(np.float32)    # [hl, 128, 32, 64]
        a = a.reshape(HC, 128, B, NCK, HD)           # hl, p, b, blk16, d
        a = a.transpose(2, 0, 3, 1, 4).reshape(B, HC, N, HD)
        vh[:, HC * c:HC * (c + 1)] = a
    out = np.concatenate(outs, axis=0).reshape(B, N, D)
    return out, kh, vh


def run(in_maps, trace=False):
    nc = _get_nc()
    return bass_utils.run_bass_kernel_spmd(
        nc, in_maps, core_ids=list(range(W)), trace=trace)


def kernel(**inputs):
    in_maps = make_in_maps(**inputs)
    res = run(in_maps, trace=False)
    return gather_outputs(res.results)
